# revision 10
# baseline (speedup 1.0000x reference)
"""BinsChamferLoss Trainium2 kernel.

Math (per batch b):
    centers c_p = 0.5*(bins[p] + bins[p+1]),  p in [0, 256)
    targets t_m = depth map pixels,           m in [0, 76800)
    out = sum_b sum_p min_m |c_p - t_m|

Sharding: data-parallel over the batch dim -- batch b on core b (8 cores).
Each core computes its batch's partial sum; the host sums the 8 scalars.

Device algorithm (per core):
  - Targets are split into a 4-term bf16 telescoping decomposition
    (t = hi + mid + lo + lo2, residual < 2^-36 relative), so the PE can
    broadcast them across all 128 partitions at full bf16 rate: a 4-row
    matmul against a ones stationary reconstructs t in fp32 PSUM exactly.
  - ACT computes |1024*t - 1024*c_p| from PSUM with the per-partition bias
    slot (queries live one-per-partition), writing fp16. The 1024 scaling
    keeps all distances in fp16 normal range.
  - DVE folds the running min in fp16 at 2x rate.
  - Epilogue: free-dim min-reduce, cross-partition sum via a ones matmul.
"""

import numpy as np

import concourse.bacc as bacc
import concourse.bass as bass
import concourse.mybir as mybir
import concourse.tile as tile
from concourse import bass_utils

F32 = mybir.dt.float32
F16 = mybir.dt.float16
BF16 = mybir.dt.bfloat16

B = 8
P = 256
M = 240 * 320  # 76800 targets per batch
CHUNK = 512    # matmul moving free dim (one PSUM bank)
GROUP = 1536   # PSUM group = 3 chunks
NSPLIT = 4     # bf16 telescoping terms
SCALE = 1024.0
BIG16 = 60000.0


def _build(m=M, reps=1):
    assert m % 3 == 0 and (m // 3) % CHUNK == 0 and m % GROUP == 0
    nc = bacc.Bacc("TRN2", target_bir_lowering=False, debug=False, enable_asserts=False)
    bins_t = nc.dram_tensor("bins", [P + 1], F32, kind="ExternalInput")
    tgt_t = nc.dram_tensor("targets", [m], F32, kind="ExternalInput")
    out_t = nc.dram_tensor("out", [1, 1], F32, kind="ExternalOutput")

    with tile.TileContext(nc) as tc:
        _body(tc, bins_t.ap(), tgt_t.ap(), out_t.ap(), m, reps)
    nc.compile()
    return nc


def _body(tc, bins, tgt, out, m, reps=1):
    nc = tc.nc
    ngroups = m // GROUP
    cpg = GROUP // CHUNK   # chunks per group
    mblock = m // 3        # targets per 32-aligned block
    nsp = NSPLIT
    # split-stage layout: [96, m/96]
    scols = m // 96

    with (
        tc.tile_pool(name="singles", bufs=1) as singles,
        tc.tile_pool(name="psum", bufs=2, space="PSUM") as psum_pool,
        tc.tile_pool(name="psum_small", bufs=1, space="PSUM") as psum_small,
        tc.tile_pool(name="dtiles", bufs=3) as dtiles,
    ):
        # --- queries: one per partition, two halves in two columns ---
        b0 = singles.tile([128, 2], F32)
        b1 = singles.tile([128, 2], F32)
        nc.sync.dma_start(out=b0[:], in_=bins[0:P].rearrange("(h p) -> p h", p=128))
        nc.sync.dma_start(out=b1[:], in_=bins[1 : P + 1].rearrange("(h p) -> p h", p=128))
        # negq = -SCALE * centers = -SCALE/2 * (b0 + b1)
        negq = singles.tile([128, 2], F32)
        nc.vector.tensor_tensor(negq[:], b0[:], b1[:], op=mybir.AluOpType.add)
        nc.vector.tensor_scalar_mul(negq[:], negq[:], -0.5 * SCALE)

        # --- load + bf16-telescope the targets in a [96, scols] layout ---
        t32 = singles.tile([96, scols], F32)
        nc.sync.dma_start(out=t32[:], in_=tgt.rearrange("(p f) -> p f", p=96))
        pieces = []
        rem = t32
        for k in range(nsp):
            pc = singles.tile([96, scols], BF16, tag=f"piece{k}")
            nc.vector.tensor_copy(pc[:], rem[:])
            if k < nsp - 1:
                nrem = singles.tile([96, scols], F32, tag=f"rem{k}")
                nc.vector.tensor_tensor(nrem[:], rem[:], pc[:], op=mybir.AluOpType.subtract)
                rem = nrem
            pieces.append(pc)

        # --- rearrange pieces into matmul rhs rows at bases {0, 32, 64} ---
        # rhs rows base+k hold piece k of the block's mblock targets.
        rhs = singles.tile([64 + nsp, mblock], BF16)
        for blk in range(3):
            for k in range(nsp):
                nc.sync.dma_start(
                    out=rhs[32 * blk + k : 32 * blk + k + 1, :],
                    in_=pieces[k][32 * blk : 32 * blk + 32, :],
                )

        # --- ones stationary at each base ---
        ones_s = singles.tile([64 + nsp, 128], BF16)
        for blk in range(3):
            nc.vector.memset(ones_s[32 * blk : 32 * blk + nsp, :], 1.0)

        # --- ones column for the final cross-partition sum ---
        ones_p = singles.tile([128, 1], F32)
        nc.vector.memset(ones_p[:], 1.0)

        # --- fp16 running minima, one tile per query half ---
        accs = []
        for h in range(2):
            a = singles.tile([128, GROUP], F16, tag=f"acc{h}")
            nc.vector.memset(a[:], BIG16)
            accs.append(a)

        # --- main loop (repeated `reps` times for delta-timing) ---
        for _rep in range(reps):
          for g in range(ngroups):
            pt = psum_pool.tile([128, GROUP], F32)
            for k in range(cpg):
                off = (g * cpg + k) * CHUNK
                blk, cc = divmod(off, mblock)
                nc.tensor.matmul(
                    pt[:, k * CHUNK : (k + 1) * CHUNK],
                    lhsT=ones_s[32 * blk : 32 * blk + nsp, :],
                    rhs=rhs[32 * blk : 32 * blk + nsp, cc : cc + CHUNK],
                    start=True,
                    stop=True,
                )
            for h in range(2):
                d16 = dtiles.tile([128, GROUP], F16)
                nc.scalar.activation(
                    d16[:],
                    pt[:],
                    mybir.ActivationFunctionType.Abs,
                    bias=negq[:, h : h + 1],
                    scale=SCALE,
                )
                nc.vector.tensor_tensor(accs[h][:], accs[h][:], d16[:], op=mybir.AluOpType.min)

        # --- epilogue: min over free dim, then sum the 256 minima ---
        mins = singles.tile([128, 2], F32)
        for h in range(2):
            nc.vector.tensor_reduce(
                mins[:, h : h + 1], accs[h][:], axis=mybir.AxisListType.X, op=mybir.AluOpType.min
            )
        ps = psum_small.tile([1, 2], F32)
        nc.tensor.matmul(ps[:], lhsT=ones_p[:], rhs=mins[:], start=True, stop=True)
        tot = singles.tile([1, 1], F32)
        nc.vector.tensor_reduce(
            tot[:], ps[:], axis=mybir.AxisListType.X, op=mybir.AluOpType.add
        )
        nc.vector.tensor_scalar_mul(tot[:], tot[:], 1.0 / SCALE)
        nc.sync.dma_start(out=out[:], in_=tot[:])


_nc_cache = {}


def _get_nc(reps=1):
    key = ("nc", reps)
    if key not in _nc_cache:
        _nc_cache[key] = _build(reps=reps)
    return _nc_cache[key]


LAST_EXEC_NS = None


def kernel(bins: np.ndarray, target_depth_maps: np.ndarray, trace: bool = False, reps: int = 1) -> np.ndarray:
    global LAST_EXEC_NS
    bins = np.ascontiguousarray(np.asarray(bins, dtype=np.float32))
    tgts = np.ascontiguousarray(
        np.asarray(target_depth_maps, dtype=np.float32).reshape(B, M)
    )
    assert bins.shape == (B, P + 1)

    nc = _get_nc(reps)
    in_maps = [{"bins": bins[i], "targets": tgts[i]} for i in range(B)]
    res = bass_utils.run_bass_kernel_spmd(nc, in_maps, core_ids=list(range(B)), trace=trace)
    LAST_EXEC_NS = res.exec_time_ns
    partials = np.array([res.results[i]["out"][0, 0] for i in range(B)], dtype=np.float32)
    return np.float32(partials.sum())


# revision 18
# speedup vs baseline: 228.0822x; 228.0822x over previous
"""BinsChamferLoss Trainium2 kernel.

Math (per batch b):
    centers c_p = 0.5*(bins[p] + bins[p+1]),  p in [0, 256)
    targets t_m = depth map pixels,           m in [0, 76800)
    out = sum_b sum_p min_m |c_p - t_m|

Sharding: data-parallel over the batch dim -- batch b on core b (8 cores).
Each core computes its batch's partial sum; the host sums the 8 scalars.

Device algorithm (per core):
  - Targets are split into a 4-term bf16 telescoping decomposition
    (t = hi + mid + lo + lo2, residual < 2^-36 relative), so the PE can
    broadcast them across all 128 partitions at full bf16 rate: a 4-row
    matmul against a ones stationary reconstructs t in fp32 PSUM exactly.
  - ACT computes |1024*t - 1024*c_p| from PSUM with the per-partition bias
    slot (queries live one-per-partition), writing fp16. The 1024 scaling
    keeps all distances in fp16 normal range.
  - DVE folds the running min in fp16 at 2x rate.
  - Epilogue: free-dim min-reduce, cross-partition sum via a ones matmul.
"""

import numpy as np

import concourse.bacc as bacc
import concourse.bass as bass
import concourse.mybir as mybir
import concourse.tile as tile
from concourse import bass_utils

F32 = mybir.dt.float32
F16 = mybir.dt.float16
BF16 = mybir.dt.bfloat16

B = 8
P = 256
M = 240 * 320  # 76800 targets per batch
CHUNK = 512    # matmul moving free dim (one PSUM bank)
GROUP = 1536   # PSUM group = 3 chunks
NSPLIT = 4     # bf16 telescoping terms
SCALE = 1024.0
BIG16 = 60000.0


def _build(m=M, reps=1, parts="full"):
    assert m % 3 == 0 and (m // 3) % CHUNK == 0 and m % GROUP == 0
    nc = bacc.Bacc("TRN2", target_bir_lowering=False, debug=False, enable_asserts=False)
    bins_t = nc.dram_tensor("bins", [P + 1], F32, kind="ExternalInput")
    tgt_t = nc.dram_tensor("targets", [m], F32, kind="ExternalInput")
    out_t = nc.dram_tensor("out", [1, 1], F32, kind="ExternalOutput")

    with tile.TileContext(nc) as tc:
        if parts.startswith("v2"):
            ttr_period = int(parts[3:]) if len(parts) > 3 else 4
            _body2(tc, bins_t.ap(), tgt_t.ap(), out_t.ap(), m, reps, ttr_period)
        elif parts.startswith("v15"):
            ttr_period = int(parts[4:]) if len(parts) > 4 else 5
            _body(
                tc, bins_t.ap(), tgt_t.ap(), out_t.ap(), m, reps, "full",
                ttr_period=ttr_period,
            )
        else:
            _body(tc, bins_t.ap(), tgt_t.ap(), out_t.ap(), m, reps, parts)
    nc.compile()
    return nc


def _telescope(nc, pool, src, shape, nterms, tag):
    """Split fp32 `src` into `nterms` bf16 tiles summing to it (to ~2^-36)."""
    pieces = []
    rem = src
    for k in range(nterms):
        pc = pool.tile(shape, BF16, tag=f"{tag}p{k}")
        nc.vector.tensor_copy(pc[:], rem[:])
        if k < nterms - 1:
            nr = pool.tile(shape, F32, tag=f"{tag}r{k}")
            nc.vector.tensor_tensor(nr[:], rem[:], pc[:], op=mybir.AluOpType.subtract)
            rem = nr
        pieces.append(pc)
    return pieces


def _body2(tc, bins, tgt, out, m, reps=1, ttr_period=4):
    """Diff-matmul variant: PSUM holds (t - q) per (group, half) via an 8-row
    bf16 matmul (4 ones rows paired with -q pieces + 4 t-piece rows paired
    with ones). Most units: ACT Abs(scale)->fp16 + DVE tt-min (2x). Every
    `ttr_period`-th unit: fused fp32-exact DVE tensor_tensor_reduce with a
    chained running min."""
    nc = tc.nc
    ngroups = m // GROUP
    cpg = GROUP // CHUNK
    mblock = m // 3
    nsp = NSPLIT
    scols = m // 96
    BIGF = 3.0e38

    with (
        tc.tile_pool(name="singles", bufs=1) as singles,
        tc.tile_pool(name="psum", bufs=2, space="PSUM") as psum_pool,
        tc.tile_pool(name="psum_small", bufs=1, space="PSUM") as psum_small,
        tc.tile_pool(name="dtiles", bufs=3) as dtiles,
        tc.tile_pool(name="dscr", bufs=2) as dscrp,
        tc.tile_pool(name="chains", bufs=4) as chains_pool,
    ):
        # --- -centers in row layout [1, 256], telescoped to bf16 ---
        b0r = singles.tile([1, P], F32)
        b1r = singles.tile([1, P], F32)
        nc.sync.dma_start(out=b0r[:], in_=bins[0:P].rearrange("(r f) -> r f", r=1))
        nc.sync.dma_start(out=b1r[:], in_=bins[1 : P + 1].rearrange("(r f) -> r f", r=1))
        negqr = singles.tile([1, P], F32)
        nc.vector.tensor_tensor(negqr[:], b0r[:], b1r[:], op=mybir.AluOpType.add)
        nc.vector.tensor_scalar_mul(negqr[:], negqr[:], -0.5)
        nqp = _telescope(nc, singles, negqr, [1, P], nsp, "nq")
        ones_row = singles.tile([1, 128], BF16)
        nc.vector.memset(ones_row[:], 1.0)

        # --- stage lhsT rows via DRAM (DVE cannot write odd partitions) ---
        stage = nc.dram_tensor("lhsT_stage", [2, 2 * nsp, 128], BF16, kind="Internal")
        stage_ap = stage.ap()
        for h in range(2):
            for k in range(nsp):
                nc.sync.dma_start(
                    out=stage_ap[h, k, :], in_=nqp[k][0:1, 128 * h : 128 * (h + 1)]
                )
                nc.sync.dma_start(out=stage_ap[h, nsp + k, :], in_=ones_row[:])
        lhsTs = []
        for h in range(2):
            lt = singles.tile([64 + 2 * nsp, 128], BF16, tag=f"lt2_{h}")
            for blk in range(3):
                nc.sync.dma_start(
                    out=lt[32 * blk : 32 * blk + 2 * nsp, :], in_=stage_ap[h, :, :]
                )
            lhsTs.append(lt)

        # --- targets: load + telescope ---
        t32 = singles.tile([96, scols], F32)
        nc.sync.dma_start(out=t32[:], in_=tgt.rearrange("(p f) -> p f", p=96))
        pieces = _telescope(nc, singles, t32, [96, scols], nsp, "t")

        # --- rhs rows: [ones x nsp | t-pieces x nsp] per 32-base ---
        rhs = singles.tile([64 + 2 * nsp, mblock], BF16)
        for blk in range(3):
            nc.vector.memset(rhs[32 * blk : 32 * blk + nsp, :], 1.0)
            for k in range(nsp):
                nc.sync.dma_start(
                    out=rhs[32 * blk + nsp + k : 32 * blk + nsp + k + 1, :],
                    in_=pieces[k][32 * blk : 32 * blk + 32, :],
                )

        ones_p = singles.tile([128, 1], F32)
        nc.vector.memset(ones_p[:], 1.0)

        accs = []
        for h in range(2):
            a = singles.tile([128, GROUP], F16, tag=f"acc{h}")
            nc.vector.memset(a[:], BIG16)
            accs.append(a)
        chain = [None, None]

        # --- main loop ---
        unit = 0
        for _rep in range(reps):
            for g in range(ngroups):
                for h in range(2):
                    pt = psum_pool.tile([128, GROUP], F32)
                    for k in range(cpg):
                        off = (g * cpg + k) * CHUNK
                        blk, cc = divmod(off, mblock)
                        nc.tensor.matmul(
                            pt[:, k * CHUNK : (k + 1) * CHUNK],
                            lhsT=lhsTs[h][32 * blk : 32 * blk + 2 * nsp, :],
                            rhs=rhs[32 * blk : 32 * blk + 2 * nsp, cc : cc + CHUNK],
                            start=True,
                            stop=True,
                        )
                    if ttr_period and (unit % ttr_period == ttr_period - 1):
                        # fused fp32-exact: running min of SQUARED distances
                        dscr_t = dscrp.tile([128, GROUP], F32)
                        newc = chains_pool.tile([128, 1], F32)
                        init = BIGF if chain[h] is None else chain[h][:]
                        nc.vector.tensor_tensor_reduce(
                            out=dscr_t[:],
                            in0=pt[:],
                            in1=pt[:],
                            scale=1.0,
                            scalar=init,
                            op0=mybir.AluOpType.mult,
                            op1=mybir.AluOpType.min,
                            accum_out=newc[:],
                        )
                        chain[h] = newc
                    else:
                        d16 = dtiles.tile([128, GROUP], F16)
                        nc.scalar.activation(
                            d16[:], pt[:], mybir.ActivationFunctionType.Abs, scale=SCALE
                        )
                        nc.vector.tensor_tensor(
                            accs[h][:], accs[h][:], d16[:], op=mybir.AluOpType.min
                        )
                    unit += 1

        # --- epilogue ---
        mins = singles.tile([128, 2], F32)
        for h in range(2):
            m16 = singles.tile([128, 1], F32, tag=f"m16_{h}")
            nc.vector.tensor_reduce(
                m16[:], accs[h][:], axis=mybir.AxisListType.X, op=mybir.AluOpType.min
            )
            nc.vector.tensor_scalar_mul(m16[:], m16[:], 1.0 / SCALE)
            if chain[h] is not None:
                # chain holds min d^2; sqrt via ACT + one Newton step (ACT
                # sqrt alone has a loose ULP budget).
                y0 = singles.tile([128, 1], F32, tag=f"y0_{h}")
                nc.scalar.activation(
                    y0[:], chain[h][:], mybir.ActivationFunctionType.Sqrt
                )
                nc.vector.tensor_scalar_max(y0[:], y0[:], 1.0e-30)
                qt = singles.tile([128, 1], F32, tag=f"qt_{h}")
                rc = singles.tile([128, 1], F32, tag=f"rc_{h}")
                nc.vector.reciprocal(rc[:], y0[:])
                nc.vector.tensor_tensor(
                    qt[:], chain[h][:], rc[:], op=mybir.AluOpType.mult
                )
                nc.vector.tensor_tensor(qt[:], qt[:], y0[:], op=mybir.AluOpType.add)
                nc.vector.tensor_scalar_mul(qt[:], qt[:], 0.5)
                nc.vector.tensor_tensor(
                    mins[:, h : h + 1], m16[:], qt[:], op=mybir.AluOpType.min
                )
            else:
                nc.vector.tensor_copy(mins[:, h : h + 1], m16[:])
        ps = psum_small.tile([1, 2], F32)
        nc.tensor.matmul(ps[:], lhsT=ones_p[:], rhs=mins[:], start=True, stop=True)
        tot = singles.tile([1, 1], F32)
        nc.vector.tensor_reduce(
            tot[:], ps[:], axis=mybir.AxisListType.X, op=mybir.AluOpType.add
        )
        nc.sync.dma_start(out=out[:], in_=tot[:])


def _body(tc, bins, tgt, out, m, reps=1, parts="full", ttr_period=0):
    nc = tc.nc
    ngroups = m // GROUP
    cpg = GROUP // CHUNK   # chunks per group
    mblock = m // 3        # targets per 32-aligned block
    nsp = NSPLIT
    # split-stage layout: [96, m/96]
    scols = m // 96

    with (
        tc.tile_pool(name="singles", bufs=1) as singles,
        tc.tile_pool(name="psum", bufs=2, space="PSUM") as psum_pool,
        tc.tile_pool(name="psum_small", bufs=1, space="PSUM") as psum_small,
        tc.tile_pool(name="dtiles", bufs=3) as dtiles,
    ):
        # --- queries: one per partition, two halves in two columns ---
        b0 = singles.tile([128, 2], F32)
        b1 = singles.tile([128, 2], F32)
        nc.sync.dma_start(out=b0[:], in_=bins[0:P].rearrange("(h p) -> p h", p=128))
        nc.sync.dma_start(out=b1[:], in_=bins[1 : P + 1].rearrange("(h p) -> p h", p=128))
        # negqU = -centers; negq = -SCALE * centers
        negqU = singles.tile([128, 2], F32)
        nc.vector.tensor_tensor(negqU[:], b0[:], b1[:], op=mybir.AluOpType.add)
        nc.vector.tensor_scalar_mul(negqU[:], negqU[:], -0.5)
        negq = singles.tile([128, 2], F32)
        nc.vector.tensor_scalar_mul(negq[:], negqU[:], SCALE)

        # --- load + bf16-telescope the targets in a [96, scols] layout ---
        t32 = singles.tile([96, scols], F32)
        nc.sync.dma_start(out=t32[:], in_=tgt.rearrange("(p f) -> p f", p=96))
        pieces = []
        rem = t32
        for k in range(nsp):
            pc = singles.tile([96, scols], BF16, tag=f"piece{k}")
            nc.vector.tensor_copy(pc[:], rem[:])
            if k < nsp - 1:
                nrem = singles.tile([96, scols], F32, tag=f"rem{k}")
                nc.vector.tensor_tensor(nrem[:], rem[:], pc[:], op=mybir.AluOpType.subtract)
                rem = nrem
            pieces.append(pc)

        # --- rearrange pieces into matmul rhs rows at bases {0, 32, 64} ---
        # rhs rows base+k hold piece k of the block's mblock targets.
        rhs = singles.tile([64 + nsp, mblock], BF16)
        for blk in range(3):
            for k in range(nsp):
                nc.sync.dma_start(
                    out=rhs[32 * blk + k : 32 * blk + k + 1, :],
                    in_=pieces[k][32 * blk : 32 * blk + 32, :],
                )

        # --- ones stationary at each base ---
        ones_s = singles.tile([64 + nsp, 128], BF16)
        for blk in range(3):
            nc.vector.memset(ones_s[32 * blk : 32 * blk + nsp, :], 1.0)

        # --- ones column for the final cross-partition sum ---
        ones_p = singles.tile([128, 1], F32)
        nc.vector.memset(ones_p[:], 1.0)

        # --- fp16 running minima, one tile per query half ---
        accs = []
        for h in range(2):
            a = singles.tile([128, GROUP], F16, tag=f"acc{h}")
            nc.vector.memset(a[:], BIG16)
            accs.append(a)
        chain = [None, None]
        BIGF = 3.0e38

        with tc.tile_pool(name="dscr", bufs=2) as dscrp, tc.tile_pool(
            name="chains", bufs=4
        ) as chains_pool:
            # --- main loop (repeated `reps` times for delta-timing) ---
            unit = 0
            for _rep in range(reps):
              for g in range(ngroups):
                pt = psum_pool.tile([128, GROUP], F32)
                for k in range(cpg):
                    off = (g * cpg + k) * CHUNK
                    blk, cc = divmod(off, mblock)
                    nc.tensor.matmul(
                        pt[:, k * CHUNK : (k + 1) * CHUNK],
                        lhsT=ones_s[32 * blk : 32 * blk + nsp, :],
                        rhs=rhs[32 * blk : 32 * blk + nsp, cc : cc + CHUNK],
                        start=True,
                        stop=True,
                    )
                if parts == "mm":
                    continue
                for h in range(2):
                    if ttr_period and (unit % ttr_period == ttr_period - 1):
                        # DVE fp32-exact: diff then fused square+min-reduce
                        d32 = dscrp.tile([128, GROUP], F32, tag="d32")
                        nc.vector.tensor_scalar(
                            d32[:], pt[:], negqU[:, h : h + 1], None,
                            op0=mybir.AluOpType.add,
                        )
                        o32 = dscrp.tile([128, GROUP], F32, tag="o32")
                        newc = chains_pool.tile([128, 1], F32)
                        init = BIGF if chain[h] is None else chain[h][:]
                        nc.vector.tensor_tensor_reduce(
                            out=o32[:], in0=d32[:], in1=d32[:], scale=1.0,
                            scalar=init, op0=mybir.AluOpType.mult,
                            op1=mybir.AluOpType.min, accum_out=newc[:],
                        )
                        chain[h] = newc
                    else:
                        d16 = dtiles.tile([128, GROUP], F16)
                        nc.scalar.activation(
                            d16[:],
                            pt[:],
                            mybir.ActivationFunctionType.Abs,
                            bias=negq[:, h : h + 1],
                            scale=SCALE,
                        )
                        if parts == "full":
                            nc.vector.tensor_tensor(accs[h][:], accs[h][:], d16[:], op=mybir.AluOpType.min)
                    unit += 1

            # --- epilogue: min over free dim, then sum the 256 minima ---
            mins = singles.tile([128, 2], F32)
            for h in range(2):
                m16 = singles.tile([128, 1], F32, tag=f"m16_{h}")
                nc.vector.tensor_reduce(
                    m16[:], accs[h][:], axis=mybir.AxisListType.X, op=mybir.AluOpType.min
                )
                nc.vector.tensor_scalar_mul(m16[:], m16[:], 1.0 / SCALE)
                if chain[h] is not None:
                    y0 = singles.tile([128, 1], F32, tag=f"y0_{h}")
                    nc.scalar.activation(
                        y0[:], chain[h][:], mybir.ActivationFunctionType.Sqrt
                    )
                    nc.vector.tensor_scalar_max(y0[:], y0[:], 1.0e-30)
                    qt = singles.tile([128, 1], F32, tag=f"qt_{h}")
                    rc = singles.tile([128, 1], F32, tag=f"rc_{h}")
                    nc.vector.reciprocal(rc[:], y0[:])
                    nc.vector.tensor_tensor(
                        qt[:], chain[h][:], rc[:], op=mybir.AluOpType.mult
                    )
                    nc.vector.tensor_tensor(qt[:], qt[:], y0[:], op=mybir.AluOpType.add)
                    nc.vector.tensor_scalar_mul(qt[:], qt[:], 0.5)
                    nc.vector.tensor_tensor(
                        mins[:, h : h + 1], m16[:], qt[:], op=mybir.AluOpType.min
                    )
                else:
                    nc.vector.tensor_copy(mins[:, h : h + 1], m16[:])
            ps = psum_small.tile([1, 2], F32)
            nc.tensor.matmul(ps[:], lhsT=ones_p[:], rhs=mins[:], start=True, stop=True)
            tot = singles.tile([1, 1], F32)
            nc.vector.tensor_reduce(
                tot[:], ps[:], axis=mybir.AxisListType.X, op=mybir.AluOpType.add
            )
            nc.sync.dma_start(out=out[:], in_=tot[:])


_nc_cache = {}


def _get_nc(reps=1, parts="full"):
    key = ("nc", reps, parts)
    if key not in _nc_cache:
        _nc_cache[key] = _build(reps=reps, parts=parts)
    return _nc_cache[key]


LAST_EXEC_NS = None


def kernel(bins: np.ndarray, target_depth_maps: np.ndarray, trace: bool = False, reps: int = 1, parts: str = "full") -> np.ndarray:
    global LAST_EXEC_NS
    bins = np.ascontiguousarray(np.asarray(bins, dtype=np.float32))
    tgts = np.ascontiguousarray(
        np.asarray(target_depth_maps, dtype=np.float32).reshape(B, M)
    )
    assert bins.shape == (B, P + 1)

    nc = _get_nc(reps, parts)
    in_maps = [{"bins": bins[i], "targets": tgts[i]} for i in range(B)]
    res = bass_utils.run_bass_kernel_spmd(nc, in_maps, core_ids=list(range(B)), trace=trace)
    LAST_EXEC_NS = res.exec_time_ns
    partials = np.array([res.results[i]["out"][0, 0] for i in range(B)], dtype=np.float32)
    return np.float32(partials.sum())
